# revision 46
# baseline (speedup 1.0000x reference)
"""Fused LN + QKV + RoPE + attention + out-proj Trainium2 kernel (v2).

Shapes (hardcoded from the problem spec):
  x [4, 2048, 512] fp32, w_qkv [512, 1536], w_out [512, 512],
  ln_gamma/ln_beta/b_out [512]. 8 heads of 64. Output [4, 2048, 512].

Sharding: 8 cores = 4 batches x 2 head-groups (4 heads each). Each core
computes a w_out row-split partial output for its batch; the host sums
the two partials per batch and adds b_out.

Design notes (all matmul operands bf16, fp32 PSUM):
- LayerNorm via bn_stats/bn_aggr on DVE, affine on the Pool engine,
  gamma folded into the weights host-side.
- xn transposed feature-major via PE (bf16 identity matmuls).
- Q/K projections: 4 M-tiles (q01 q23 k01 k23); the RoPE roll(q,1,-1)
  is a partition permutation within each head's 64 features, done on
  the PE with a 128x128 permutation matrix against the raw Q/K evacuated
  to SBUF (cheaper than doubling the QKV matmuls with pre-rolled
  weights). Combine (x*cos + roll(x)*sin) on DVE, final add on Pool.
- V is computed token-major directly (lhsT = xn^T tile, rhs = Wv) so no
  PE transpose of V is needed; a ones column rides along for the softmax
  denominator.
- Scores S^T = k^T.T @ q^T per (head, token-tile); softmax exp runs on
  the Activation engine for most tiles and as a one-instruction
  Schraudolph fast-exp (int16 bitcast to bf16) on the Vector engine for
  a calibrated subset, balancing the two engines.
- PV uses the token-major orientation (lhsT = P chunk, rhs = V_aug)
  which keeps all 128 PE output columns busy (65-row streams) and makes
  the softmax denominator a per-partition scalar: normalization is a
  tiny reciprocal + one broadcast multiply.
- Attention output transposed back feature-major on PE, out-proj in
  bf16, fp32 partial DMA'd out.
"""

import numpy as np

import concourse.bass as bass
import concourse.tile as tile
from concourse import mybir
from concourse.bass_utils import run_bass_kernel_spmd

F32 = mybir.dt.float32
BF16 = mybir.dt.bfloat16
I16 = mybir.dt.int16
AX = mybir.AxisListType
OP = mybir.AluOpType
ACT = mybir.ActivationFunctionType

B, N, D = 4, 2048, 512
HEADS, DH = 8, 64
HPC = 4            # heads per core
EPS = 1e-5
NT = N // 128      # 16 token tiles
KT = D // 128      # 4 feature tiles
SCALE = float(DH) ** -0.5
# fast-exp: bf16(bitcast16(int16(s*FE_A + FE_B))) ~= exp(s*SCALE)
FE_A = float(128.0 / np.log(2.0) * SCALE)
FE_B = float(127.0 * 128.0 - 7.75)


def _split_multiwait(nc):
    """Insert NoOps so no instruction carries more than one sem wait.

    The pinned walrus rejects >1 sync wait per instruction
    (setupSyncWait "Too many sync wait commands"). Waits are a
    conjunction, so hoisting all but the last onto same-engine NoOps
    immediately before the instruction is equivalent.
    """
    ctr = 0
    for fn in nc.m.functions:
        for blk in fn.blocks:
            insts = blk.instructions
            idx = 0
            while idx < len(insts):
                inst = insts[idx]
                si = inst.sync_info
                if si is not None and len(si.on_wait) > 1:
                    waits = list(si.on_wait)
                    for w in waits[:-1]:
                        nop = mybir.InstNoOp(name=f"SWNOP-{ctr}", ins=[], outs=[])
                        ctr += 1
                        nop.engine = inst.engine
                        nop.sync_info = mybir.SyncInfo(on_wait=[w], on_update=[])
                        insts.insert(idx, nop)
                        idx += 1
                    inst.sync_info = mybir.SyncInfo(
                        on_wait=[waits[-1]], on_update=list(si.on_update)
                    )
                idx += 1


def _dve_exp(h, mt, h2):
    """Which score tiles get the DVE fast-exp (balance Act vs DVE)."""
    return (2 * mt + h2) % 8 in (1, 4, 6)


def build_nc(loops=1, split_mw=True, stages='abcd', has_beta=False):
    from contextlib import ExitStack

    nc = bass.Bass("TRN2", target_bir_lowering=False, num_devices=8)

    x_nat = nc.dram_tensor("x_nat", [N, D], F32, kind="ExternalInput")
    # gamma-folded q/k weights, 4 M-tiles of 128 cols: q01 q23 k01 k23
    wqk = nc.dram_tensor("wqk", [D, 512], BF16, kind="ExternalInput")
    beta_qk = nc.dram_tensor("beta_qk", [128, 8], F32, kind="ExternalInput")
    wv = nc.dram_tensor("wv", [D, 256], BF16, kind="ExternalInput")
    beta_v = nc.dram_tensor("beta_v", [128, 256], F32, kind="ExternalInput")
    wout = nc.dram_tensor("wout", [HPC * DH, D], BF16, kind="ExternalInput")
    cos_t = nc.dram_tensor("cos_t", [128, N], BF16, kind="ExternalInput")
    sin_t = nc.dram_tensor("sin_t", [128, N], BF16, kind="ExternalInput")
    ident = nc.dram_tensor("ident", [128, 128], BF16, kind="ExternalInput")
    rollm = nc.dram_tensor("rollm", [128, 128], BF16, kind="ExternalInput")
    y = nc.dram_tensor("y", [D, N], F32, kind="ExternalOutput")

    with tile.TileContext(nc) as tc:
      for _loop in range(loops):
        with ExitStack() as ctx:
          const = ctx.enter_context(tc.tile_pool(name="const", bufs=1))
          xnT_p = ctx.enter_context(tc.tile_pool(name="xnT", bufs=1))
          qks_p = ctx.enter_context(tc.tile_pool(name="qks", bufs=1))
          va_p = ctx.enter_context(tc.tile_pool(name="va", bufs=1))
          outn_p = ctx.enter_context(tc.tile_pool(name="outn", bufs=1))
          P_p = ctx.enter_context(tc.tile_pool(name="Pp", bufs=42))
          st_p = ctx.enter_context(tc.tile_pool(name="st", bufs=8))
          xnb_p = ctx.enter_context(tc.tile_pool(name="xnb", bufs=4))
          qsb_p = ctx.enter_context(tc.tile_pool(name="qsb", bufs=3))
          tq_p = ctx.enter_context(tc.tile_pool(name="tq", bufs=2))
          on_p = ctx.enter_context(tc.tile_pool(name="on", bufs=2))
          rb_p = ctx.enter_context(tc.tile_pool(name="rb", bufs=4))
          # PSUM: one shared carve for all stages (8 banks total)
          sm_ps = ctx.enter_context(
              tc.tile_pool(name="smp", bufs=2, space="PSUM"))
          big_ps = ctx.enter_context(
              tc.tile_pool(name="bigp", bufs=3, space="PSUM"))

          _bigc = [0]

          def big_tile():
              _bigc[0] += 1
              bt = big_ps.tile([128, 1024], F32, tag="big",
                               name=f"bg{_bigc[0]}")
              return bt

          # x first (right side of SBUF, released after stage A)
          xin_p = tc.alloc_tile_pool(name="xin", bufs=1, side="right")
          # first token-tile in its own small DMA so the LN chain starts
          # ~3us earlier; the rest in mega-DMAs
          xt1 = xin_p.tile([128, 1, D], F32, tag="xt1")
          nc.sync.dma_start(
              xt1[:], x_nat[0:128, :].rearrange("(a p) d -> p a d", p=128))
          xt3 = xin_p.tile([128, 3, D], F32, tag="xt3")
          nc.sync.dma_start(
              xt3[:], x_nat[128:512, :].rearrange("(a p) d -> p a d", p=128))
          xt4 = []
          for blk in range(1, 4):
              t = xin_p.tile([128, 4, D], F32, tag=f"xt4_{blk}")
              nc.sync.dma_start(
                  t[:], x_nat[blk * 512:(blk + 1) * 512, :].rearrange(
                      "(a p) d -> p a d", p=128))
              xt4.append(t)

          def xt_view(tt):
              if tt == 0:
                  return xt1[:, 0, :]
              if tt < 4:
                  return xt3[:, tt - 1, :]
              return xt4[tt // 4 - 1][:, tt % 4, :]

          ident_sb = const.tile([128, 128], BF16)
          nc.sync.dma_start(ident_sb[:], ident[:, :])
          rollm_sb = const.tile([128, 128], BF16)
          nc.sync.dma_start(rollm_sb[:], rollm[:, :])
          wv_sb = const.tile([128, 4, 256], BF16)
          nc.sync.dma_start(wv_sb[:],
                            wv[:, :].rearrange("(k p) m -> p k m", p=128))
          wqk_all = const.tile([128, KT, 512], BF16)
          nc.sync.dma_start(wqk_all[:],
                            wqk[:, :].rearrange("(k p) m -> p k m", p=128))
          wqk_sb = [wqk_all[:, kt, :] for kt in range(KT)]
          eps_sb = const.tile([128, 1], F32)
          nc.vector.memset(eps_sb[:], EPS)
          beta_sb = const.tile([128, 8], F32)
          nc.sync.dma_start(beta_sb[:], beta_qk[:, :])
          bv_sb = const.tile([128, 256], F32)
          nc.sync.dma_start(bv_sb[:], beta_v[:, :])
          cos_sb = const.tile([128, N], BF16)
          nc.sync.dma_start(cos_sb[:], cos_t[:, :])
          sin_sb = const.tile([128, N], BF16)
          nc.sync.dma_start(sin_sb[:], sin_t[:, :])
          wout_all = const.tile([128, 2, D], BF16)
          nc.sync.dma_start(wout_all[:],
                            wout[:, :].rearrange("(k p) m -> p k m", p=128))
          wout_sb = [wout_all[:, kt, :] for kt in range(2)]

          # persistent activations
          xnT = xnT_p.tile([128, KT, N], BF16)       # xn^T, per k-tile
          qs = [qks_p.tile([128, N], BF16, name=f"qs{i}", tag=f"qs{i}")
                for i in range(2)]
          ks = [qks_p.tile([128, N], BF16, name=f"ks{i}", tag=f"ks{i}")
                for i in range(2)]
          va = va_p.tile([128, NT, HPC, 65], BF16)   # token-major V + ones col
          outn = outn_p.tile([128, 2, N], BF16)      # attn out, feature-major

          def b_iter(hp, m, dst_list, ch):
                  mi = m + hp
                  dst = dst_list[hp]
                  if True:
                      cs = slice(ch * 1024, (ch + 1) * 1024)
                      pq = big_tile()
                      qsb = qsb_p.tile([128, 1024], BF16, tag="qsb")
                      for nn in range(2):
                          ns = slice(ch * 1024 + nn * 512,
                                     ch * 1024 + (nn + 1) * 512)
                          hs = slice(nn * 512, (nn + 1) * 512)
                          for kt in range(KT):
                              nc.tensor.matmul(
                                  pq[:, hs],
                                  wqk_sb[kt][:, mi * 128:(mi + 1) * 128],
                                  xnT[:, kt, ns],
                                  start=(kt == 0), stop=(kt == KT - 1))
                          # evac each half immediately so the roll matmul
                          # can start while the other half is computed
                          if hp == 1 and ch == 1:
                              nc.vector.tensor_copy(qsb[:, hs], pq[:, hs])
                          else:
                              nc.scalar.activation(qsb[:, hs], pq[:, hs],
                                                   ACT.Copy)
                      pqr = big_tile()
                      for nn in range(2):
                          nc.tensor.matmul(
                              pqr[:, nn * 512:(nn + 1) * 512],
                              rollm_sb[:],
                              qsb[:, nn * 512:(nn + 1) * 512],
                              start=True, stop=True)
                      tq = tq_p.tile([128, 1024], BF16, tag="tq")
                      if has_beta:
                          nc.vector.scalar_tensor_tensor(
                              tq[:], qsb[:], beta_sb[:, mi:mi + 1],
                              cos_sb[:, cs], op0=OP.add, op1=OP.mult)
                          nc.vector.scalar_tensor_tensor(
                              dst[:, cs], pqr[:], beta_sb[:, 4 + mi:5 + mi],
                              sin_sb[:, cs], op0=OP.add, op1=OP.mult)
                      else:
                          nc.vector.tensor_tensor(
                              tq[:], qsb[:], cos_sb[:, cs], op=OP.mult)
                          nc.vector.tensor_tensor(
                              dst[:, cs], pqr[:], sin_sb[:, cs], op=OP.mult)
                      nc.gpsimd.tensor_tensor(
                          dst[:, cs], dst[:, cs], tq[:], op=OP.add)

          # ---- Stage A: LayerNorm + transpose ----
          for tt in range(NT):
              ts = slice(tt * 128, (tt + 1) * 128)
              xt = xt_view(tt)
              bn = st_p.tile([128, 6], F32, tag="bn")
              nc.vector.bn_stats(bn[:], xt)
              mv = st_p.tile([128, 2], F32, tag="mv")
              nc.vector.bn_aggr(mv[:], bn[:])
              sd = st_p.tile([128, 1], F32, tag="sd")
              nc.scalar.activation(sd[:], mv[:, 1:2], ACT.Sqrt, bias=eps_sb[:])
              rs = st_p.tile([128, 1], F32, tag="rs")
              nc.vector.reciprocal(rs[:], sd[:])
              xnb = xnb_p.tile([128, D], BF16, tag="xnb")
              nc.gpsimd.tensor_scalar(xnb[:], xt, mv[:, 0:1], rs[:],
                                      op0=OP.subtract, op1=OP.mult)
              xp = sm_ps.tile([128, KT, 128], BF16, tag="xp")
              for ft in range(KT):
                  nc.tensor.transpose(
                      xp[:, ft, :], xnb[:, ft * 128:(ft + 1) * 128],
                      ident_sb[:])
              nc.vector.tensor_copy(xnT[:, :, ts], xp[:])
              if tt == 9:
                  b_iter(0, 0, qs, 0)
              if tt == 11:
                  b_iter(0, 2, ks, 0)
          xin_p.release()

          if 'b' not in stages:
              continue
          # ---- Stage B: V (token-major) then Q/K + RoPE ----
          nc.vector.memset(va[:, :, :, 64], 1.0)
          bv_b = bv_sb[:].rearrange("p (h f) -> p h f", h=HPC)
          for tt in range(NT):
              ts = slice(tt * 128, (tt + 1) * 128)
              pvt = big_tile()
              pv = pvt[:, 0:256]
              for kt in range(KT):
                  nc.tensor.matmul(pv, xnT[:, kt, ts], wv_sb[:, kt, :],
                                   start=(kt == 0), stop=(kt == KT - 1))
              nc.vector.tensor_tensor(
                  va[:, tt, :, 0:64],
                  pv.rearrange("p (h f) -> p h f", h=HPC),
                  bv_b, op=OP.add)

          # ---- Stage C: attention ----
          P_tiles = {}
          on_tiles = [on_p.tile([128, NT, 64], BF16, tag=f"on{h % 2}",
                                name=f"on{h}")
                      for h in range(HPC)]

          def scores_mt(h, mt):
              hp, hh = h // 2, h % 2
              sl = slice(64 * hh, 64 * hh + 64)
              for h2 in range(2):
                  Pt = P_p.tile([128, 1024], BF16, tag="P",
                                name=f"P{h}_{mt}_{h2}")
                  P_tiles[(h, mt, h2)] = Pt
                  sp = big_tile()
                  for nn in range(2):
                      ns = slice(h2 * 1024 + nn * 512,
                                 h2 * 1024 + (nn + 1) * 512)
                      nc.tensor.matmul(
                          sp[:, nn * 512:(nn + 1) * 512],
                          ks[hp][sl, mt * 128:(mt + 1) * 128],
                          qs[hp][sl, ns], start=True, stop=True)
                  if _dve_exp(h, mt, h2):
                      nc.vector.tensor_scalar(
                          Pt[:].bitcast(I16), sp[:], FE_A, FE_B,
                          op0=OP.mult, op1=OP.add)
                  else:
                      nc.scalar.activation(Pt[:], sp[:], ACT.Exp,
                                           scale=SCALE)

          def pv_norm(h, g):
              # one group = 2 query tiles
              opt = sm_ps.tile([128, 2, 128], F32, tag="xp",
                               name=f"op{h}_{g}")
              for qi in range(2):
                  qt = 2 * g + qi
                  qsl = slice(qt * 128, (qt + 1) * 128)
                  for mt in range(NT):
                      nc.tensor.matmul(
                          opt[:, qi, 0:65],
                          P_tiles[(h, mt, qt // 8)][:, (qt % 8) * 128:
                                                    (qt % 8) * 128 + 128],
                          va[:, mt, h, :],
                          start=(mt == 0), stop=(mt == NT - 1))
              rb = rb_p.tile([128, 2], BF16, tag="rb")
              with nc.allow_low_precision(reason="softmax recip bf16"):
                  nc.vector.reciprocal(rb[:], opt[:, :, 64])
              ont = on_tiles[h]
              rb_b = rb[:].unsqueeze(2).broadcast_to([128, 2, 64])
              nc.vector.tensor_tensor(
                  ont[:, 2 * g:2 * g + 2, :], opt[:, :, 0:64], rb_b,
                  op=OP.mult)

          def tr_evac(h, e):
              # one evac = 4 query tiles (2 pv groups)
              hp, hh = h // 2, h % 2
              ont = on_tiles[h]
              trp = sm_ps.tile([128, KT, 128], BF16, tag="xp")
              for qi in range(4):
                  nc.tensor.transpose(
                      trp[64 * hh:64 * hh + 64, qi, :],
                      ont[:, 4 * e + qi, :], ident_sb[:])
              nc.vector.tensor_copy(
                  outn[64 * hh:64 * hh + 64, hp, e * 512:(e + 1) * 512],
                  trp[64 * hh:64 * hh + 64, :, :])

          def po_chunk(ch):
              cs = slice(ch * 1024, (ch + 1) * 1024)
              for mi in range(4):
                  po = big_tile()
                  for nn in range(2):
                      ns = slice(ch * 1024 + nn * 512,
                                 ch * 1024 + (nn + 1) * 512)
                      for kt in range(2):
                          nc.tensor.matmul(
                              po[:, nn * 512:(nn + 1) * 512],
                              wout_sb[kt][:, mi * 128:(mi + 1) * 128],
                              outn[:, kt, ns],
                              start=(kt == 0), stop=(kt == 1))
                  ye = ye_tiles[mi]
                  nc.scalar.activation(ye[:, cs], po[:], ACT.Copy)
                  nc.sync.dma_start(y[mi * 128:(mi + 1) * 128, cs], ye[:, cs])

          # Stage B emission: hp0 q/k first, then hp1 interleaved with
          # head-0 scores (Act/DVE exp overlap the hp1 projections).
          for m, dst_list in ((0, qs), (2, ks)):
              for ch in range(2):
                  if ch == 0:
                      continue  # emitted during stage A
                  b_iter(0, m, dst_list, ch)
          if 'c' not in stages:
              for m, dst_list in ((0, qs), (2, ks)):
                  for ch in range(2):
                      b_iter(1, m, dst_list, ch)
              continue
          h0mt = [0]

          def h0_scores(n):
              for _ in range(n):
                  if h0mt[0] < NT:
                      scores_mt(0, h0mt[0])
                      h0mt[0] += 1

          for m, dst_list in ((0, qs), (2, ks)):
              for ch in range(2):
                  b_iter(1, m, dst_list, ch)
                  h0_scores(2)
          h0_scores(NT)

          # software pipeline: scores(h) over mt; pv_norm(h-1) packed into
          # the first 4 steps (so P-pool slot reuse at mt>=5 only ever
          # references finished groups), tr_evac(h-1) trailing behind.
          for h in range(1, HPC):
              for mt in range(NT):
                  scores_mt(h, mt)
                  if mt < 8:
                      pv_norm(h - 1, mt)
                  if mt in (9, 11, 13, 15):
                      tr_evac(h - 1, (mt - 9) // 2)
          if 'd' in stages:
              ye_p = tc.alloc_tile_pool(name="ye", bufs=2, side="right")
              ye_tiles = [ye_p.tile([128, N], F32, tag=f"ye{mi % 2}",
                                    name=f"ye{mi}") for mi in range(4)]
          h3 = HPC - 1
          for g in range(8):
              pv_norm(h3, g)
              if g == 3:
                  tr_evac(h3, 0)
              if g == 5:
                  tr_evac(h3, 1)
                  if 'd' in stages:
                      po_chunk(0)
              if g == 7:
                  tr_evac(h3, 2)
                  tr_evac(h3, 3)
                  if 'd' in stages:
                      po_chunk(1)
          if 'd' in stages:
              ye_p.release()

    if split_mw:
        _split_multiwait(nc)
    return nc


def _host_prep(x, ln_gamma, ln_beta, w_qkv, w_out):
    """Build the 8 per-core input maps."""
    import ml_dtypes
    f32 = np.float32
    bf16 = ml_dtypes.bfloat16
    pos = np.arange(N, dtype=f32)[:, None]
    idx = np.arange(DH, dtype=f32)[None, :]
    angle = pos / (f32(10000.0) ** (idx / f32(DH)))       # [N, DH]
    cos2 = np.ascontiguousarray(np.tile(np.cos(angle).T, (2, 1))).astype(bf16)
    sin2 = np.ascontiguousarray(np.tile(np.sin(angle).T, (2, 1))).astype(bf16)
    ident = np.eye(128, dtype=f32).astype(bf16)
    # roll matrix: out[p] = in[src(p)], src = 64*hh + (f-1) % 64
    rollm = np.zeros((128, 128), dtype=f32)
    for p in range(128):
        hh, f = p // 64, p % 64
        rollm[64 * hh + (f - 1) % 64, p] = 1.0
    rollm = rollm.astype(bf16)

    wg = (w_qkv * ln_gamma[:, None]).astype(f32)          # [512, 1536]
    beta_row = (ln_beta @ w_qkv).astype(f32)              # [1536]

    def head_block(a, sec, h):    # sec 0=q 1=k 2=v, global head h
        return a[..., sec * 512 + h * DH: sec * 512 + (h + 1) * DH]

    in_maps = []
    for c in range(8):
        bi, hg = c // 2, c % 2
        hs = [4 * hg + i for i in range(HPC)]
        # M-tiles q01 q23 k01 k23 (128 cols each = 2 heads)
        mts, bcols = [], []
        for sec, p in ((0, 0), (0, 1), (1, 0), (1, 1)):
            mts.append(np.concatenate(
                [head_block(wg, sec, hs[2 * p]),
                 head_block(wg, sec, hs[2 * p + 1])], axis=1))
            bcols.append(np.concatenate(
                [head_block(beta_row, sec, hs[2 * p]),
                 head_block(beta_row, sec, hs[2 * p + 1])]))
        wqk_c = np.ascontiguousarray(
            np.concatenate(mts, axis=1)).astype(bf16)     # [512, 512]
        bcols_r = [np.concatenate([np.roll(b[0:64], 1), np.roll(b[64:128], 1)])
                   for b in bcols]
        beta_c = np.stack(bcols + bcols_r, axis=1).astype(f32)   # [128, 8]
        wv_c = np.ascontiguousarray(np.concatenate(
            [head_block(wg, 2, h) for h in hs], axis=1)).astype(bf16)
        bv_c = np.ascontiguousarray(np.tile(np.concatenate(
            [head_block(beta_row, 2, h) for h in hs])[None, :],
            (128, 1))).astype(f32)
        wout_c = np.ascontiguousarray(
            w_out[hg * 256:(hg + 1) * 256, :]).astype(bf16)
        in_maps.append({
            "x_nat": np.ascontiguousarray(x[bi], dtype=f32),
            "wqk": wqk_c,
            "beta_qk": beta_c,
            "wv": wv_c,
            "beta_v": bv_c,
            "wout": wout_c,
            "cos_t": cos2,
            "sin_t": sin2,
            "ident": ident,
            "rollm": rollm,
        })
    return in_maps


_NC = None


def kernel(x, ln_gamma, ln_beta, w_qkv, w_out, b_out, **run_kwargs):
    global _NC
    x = np.asarray(x, dtype=np.float32)
    assert x.shape == (B, N, D), x.shape
    hb = bool(np.any(np.asarray(ln_beta)))
    if _NC is None or getattr(kernel, "_hb", None) != hb:
        _NC = build_nc(has_beta=hb)
        kernel._hb = hb
    in_maps = _host_prep(np.asarray(x), np.asarray(ln_gamma),
                         np.asarray(ln_beta), np.asarray(w_qkv),
                         np.asarray(w_out))
    res = run_bass_kernel_spmd(_NC, in_maps, core_ids=list(range(8)), **run_kwargs)
    out = np.empty((B, N, D), dtype=np.float32)
    for bi in range(B):
        part = res.results[2 * bi]["y"] + res.results[2 * bi + 1]["y"]
        out[bi] = part.T + np.asarray(b_out, dtype=np.float32)
    kernel.last_results = res
    return out
